# revision 8
# baseline (speedup 1.0000x reference)
"""DSMIL bag-of-tiles kernel for 8 Trainium2 NeuronCores (Bass/Tile).

Reference computation (per slide b of B=16, N=8192 tiles, F=2048 features):
    scores = feats @ W_scores + b_scores          (masked; max + argmax over N)
    queries = feats @ W_q + b_q
    attn    = softmax(queries @ queries[argmax])  (masked)
    bag     = sum(attn * (feats @ W_mlp + b_mlp))
    out[b]  = 0.5 * (max_score + bag)

Sharding: data-parallel over slides, 2 slides per core; small weights replicated.

Device-side layout: the PE contracts the partition dimension, so the feature
dimension must sit on partitions for every heavy matmul. The host hands each
core its feats pre-transposed as xT[slide, feature, token] f32; the device
then streams featsT chunks through the PE with the (replicated, repacked)
weights stationary. Everything on device is fp32, so max/argmax/softmax match
the fp32 reference exactly (no low-precision rescue logic needed).
"""

import os
import sys
import concurrent.futures as _futures

import numpy as np

for _p in ("/opt/trn_rl_repo", "/opt/pypackages"):
    if os.path.isdir(_p) and _p not in sys.path:
        sys.path.append(_p)

import concourse.bass as bass
import concourse.tile as tile
from concourse import bacc, mybir
from concourse import bass_utils

F32 = mybir.dt.float32
U32 = mybir.dt.uint32
AF = mybir.ActivationFunctionType
OP = mybir.AluOpType

# Problem constants (hardcoded per harness contract).
B, N, IN_FEAT, META, DQ = 16, 8192, 2048, 3, 128
N_CORES = 8
SPC = B // N_CORES          # slides per core = 2
P = 128                     # partitions / contraction tile
NEG_BIG = -1.0e30


def build_nc(n_tok=N, n_feat=IN_FEAT, spc=SPC, chunk=2048, ft_bufs=6):
    """Build the per-core Bass program. Returns (nc, input name list)."""
    nk = n_feat // P
    chunk = min(chunk, n_tok)
    n_chunks = n_tok // chunk
    sub = min(1024, n_tok)          # attn/softmax sub-chunk
    n_sub = n_tok // sub
    mm = 512                        # fp32 moving-operand limit / psum bank

    nc = bacc.Bacc("TRN2", target_bir_lowering=False, debug=False)

    xT_d = nc.dram_tensor("xT", [spc, n_feat, n_tok], F32, kind="ExternalInput").ap()
    wq_d = nc.dram_tensor("wq", [P, nk * DQ], F32, kind="ExternalInput").ap()
    wsm_d = nc.dram_tensor("wsm", [P, nk * 2], F32, kind="ExternalInput").ap()
    bq_d = nc.dram_tensor("bq", [P, 1], F32, kind="ExternalInput").ap()
    bsm_d = nc.dram_tensor("bsm", [2, 1], F32, kind="ExternalInput").ap()
    mneg_d = nc.dram_tensor("mneg", [spc, n_tok], F32, kind="ExternalInput").ap()
    out_d = nc.dram_tensor("out", [1, spc], F32, kind="ExternalOutput").ap()

    with tile.TileContext(nc) as tc:
        with (
            tc.tile_pool(name="consts", bufs=1) as cpool,
            tc.tile_pool(name="ft", bufs=ft_bufs) as ftpool,
            tc.tile_pool(name="qt", bufs=1) as qtpool,
            tc.tile_pool(name="rows", bufs=1) as rpool,
            tc.tile_pool(name="tiny", bufs=2) as tpool,
            tc.tile_pool(name="psq", bufs=1, space="PSUM") as psq,
            tc.tile_pool(name="psr", bufs=1, space="PSUM") as psr,
        ):
            wq_sb = cpool.tile([P, nk * DQ], F32)
            nc.sync.dma_start(wq_sb[:], wq_d[:])
            wsm_sb = cpool.tile([P, nk * 2], F32)
            nc.sync.dma_start(wsm_sb[:], wsm_d[:])
            bq_sb = cpool.tile([P, 1], F32)
            nc.sync.dma_start(bq_sb[:], bq_d[:])
            bsm_sb = cpool.tile([2, 1], F32)
            nc.sync.dma_start(bsm_sb[:], bsm_d[:])
            out_sb = cpool.tile([1, spc], F32)

            for s in range(spc):
                # scoresT/mlpT rows [2, n_tok] and queriesT [DQ, n_tok]
                m2 = rpool.tile([2, n_tok], F32, tag="m2", name=f"m2_{s}")
                qT = qtpool.tile([DQ, n_tok], F32, tag="qt", name=f"qT{s}")

                for c in range(n_chunks):
                    c0 = c * chunk
                    qp = psq.tile([DQ, chunk], F32, tag="qp", name=f"qp{s}_{c}")
                    sp = psr.tile([2, chunk], F32, tag="rowps", name=f"sp{s}_{c}")
                    for k in range(nk):
                        ft = ftpool.tile([P, chunk], F32, tag="ft", name=f"ft{s}_{c}_{k}")
                        nc.sync.dma_start(
                            ft[:], xT_d[s, k * P : (k + 1) * P, c0 : c0 + chunk]
                        )
                        st, sp_flag = (k == 0), (k == nk - 1)
                        for j in range(chunk // mm):
                            js = slice(j * mm, (j + 1) * mm)
                            nc.tensor.matmul(
                                qp[:, js],
                                wq_sb[:, k * DQ : (k + 1) * DQ],
                                ft[:, js],
                                start=st,
                                stop=sp_flag,
                            )
                            nc.tensor.matmul(
                                sp[:, js],
                                wsm_sb[:, k * 2 : (k + 1) * 2],
                                ft[:, js],
                                start=st,
                                stop=sp_flag,
                            )
                    # evacuate PSUM (+bias) on the scalar engine
                    nc.scalar.activation(
                        qT[:, c0 : c0 + chunk], qp[:], AF.Identity, bias=bq_sb[:], scale=1.0
                    )
                    nc.scalar.activation(
                        m2[:, c0 : c0 + chunk], sp[:], AF.Identity, bias=bsm_sb[:], scale=1.0
                    )

                # mask scores row in place (chunked), then global top-1 value
                for c in range(n_chunks):
                    cs = slice(c * chunk, (c + 1) * chunk)
                    mnc = tpool.tile([1, chunk], F32, tag="mnc", name=f"mnc{s}_{c}")
                    nc.sync.dma_start(mnc[:], mneg_d[s : s + 1, cs])
                    nc.vector.tensor_tensor(m2[0:1, cs], m2[0:1, cs], mnc[:], OP.add)
                m8 = tpool.tile([1, 8], F32, tag="m8", name=f"m8_{s}")
                nc.vector.max(m8[:], m2[0:1, :])

                # q_top[d] = sum_t qT[d, t] * onehot[t], via bcast-DMA + mult + reduce
                qparts = tpool.tile([DQ, n_chunks], F32, tag="qparts", name=f"qparts{s}")
                for c in range(n_chunks):
                    cs = slice(c * chunk, (c + 1) * chunk)
                    hc = tpool.tile([1, chunk], F32, tag="hc", name=f"hc{s}_{c}")
                    nc.vector.tensor_scalar(
                        hc[:], m2[0:1, cs], m8[0:1, 0:1], None, OP.is_equal
                    )
                    hb = tpool.tile([DQ, chunk], F32, tag="hb", name=f"hb{s}_{c}")
                    nc.gpsimd.partition_broadcast(hb[:], hc[:], channels=DQ)
                    nc.vector.tensor_tensor(hb[:], hb[:], qT[:, cs], OP.mult)
                    nc.vector.tensor_reduce(
                        qparts[:, c : c + 1], hb[:], axis=mybir.AxisListType.X, op=OP.add
                    )
                qtop = tpool.tile([DQ, 1], F32, tag="qtop", name=f"qtop{s}")
                nc.vector.tensor_reduce(
                    qtop[:], qparts[:], axis=mybir.AxisListType.X, op=OP.add
                )

                # softmax reference point M0 = q_top . q_top
                m0p = psr.tile([1, 1], F32, tag="rowps", name=f"m0p{s}")
                nc.tensor.matmul(m0p[0:1, 0:1], qtop[:], qtop[:])
                negM0 = tpool.tile([1, 1], F32, tag="negm0", name=f"negM0_{s}")
                nc.vector.tensor_scalar(negM0[:], m0p[0:1, 0:1], -1.0, None, OP.mult)

                zparts = tpool.tile([1, n_sub], F32, tag="zparts", name=f"zparts{s}")
                sparts = tpool.tile([1, n_sub], F32, tag="sparts", name=f"sparts{s}")
                for q in range(n_sub):
                    q0 = q * sub
                    qs = slice(q0, q0 + sub)
                    atp = psr.tile([1, sub], F32, tag="rowps", name=f"atp{s}_{q}")
                    for j in range(sub // mm):
                        js = slice(j * mm, (j + 1) * mm)
                        nc.tensor.matmul(
                            atp[0:1, js], qtop[:], qT[:, q0 + j * mm : q0 + (j + 1) * mm]
                        )
                    mnq = tpool.tile([1, sub], F32, tag="mnq", name=f"mnq{s}_{q}")
                    nc.sync.dma_start(mnq[:], mneg_d[s : s + 1, qs])
                    al = tpool.tile([1, sub], F32, tag="al", name=f"al{s}_{q}")
                    nc.vector.scalar_tensor_tensor(
                        al[:], atp[0:1, :], 0.0, mnq[:], OP.add, OP.add
                    )
                    ex = tpool.tile([1, sub], F32, tag="ex", name=f"ex{s}_{q}")
                    nc.scalar.activation(ex[:], al[:], AF.Exp, bias=negM0[:], scale=1.0)
                    nc.vector.tensor_reduce(
                        zparts[0:1, q : q + 1], ex[:], axis=mybir.AxisListType.X, op=OP.add
                    )
                    mc = tpool.tile([1, sub], F32, tag="mc", name=f"mc{s}_{q}")
                    nc.sync.dma_start(mc[:], m2[1:2, qs])
                    nc.vector.tensor_tensor(ex[:], ex[:], mc[:], OP.mult)
                    nc.vector.tensor_reduce(
                        sparts[0:1, q : q + 1], ex[:], axis=mybir.AxisListType.X, op=OP.add
                    )

                zz = tpool.tile([1, 1], F32, tag="zz", name=f"zz{s}")
                nc.vector.tensor_reduce(zz[:], zparts[:], axis=mybir.AxisListType.X, op=OP.add)
                ss = tpool.tile([1, 1], F32, tag="ss", name=f"ss{s}")
                nc.vector.tensor_reduce(ss[:], sparts[:], axis=mybir.AxisListType.X, op=OP.add)
                rz = tpool.tile([1, 1], F32, tag="rz", name=f"rz{s}")
                nc.vector.reciprocal(rz[:], zz[:])
                bag = tpool.tile([1, 1], F32, tag="bag", name=f"bag{s}")
                nc.vector.tensor_tensor(bag[:], ss[:], rz[:], OP.mult)
                tot = tpool.tile([1, 1], F32, tag="tot", name=f"tot{s}")
                nc.vector.tensor_tensor(tot[:], bag[:], m8[0:1, 0:1], OP.add)
                nc.vector.tensor_scalar(out_sb[0:1, s : s + 1], tot[:], 0.5, None, OP.mult)

            nc.sync.dma_start(out_d[:], out_sb[:])

    nc.compile()
    in_names = ["xT", "wq", "wsm", "bq", "bsm", "mneg"]
    return nc, in_names


def pack_core_inputs(x, mask, W_scores, b_scores, W_q, b_q, W_mlp, b_mlp,
                     n_tok=N, n_feat=IN_FEAT, spc=SPC, meta=META):
    """Host-side shard + repack: per-core input dicts for run_bass_kernel_spmd."""
    x = np.asarray(x, dtype=np.float32)
    mask = np.asarray(mask).astype(bool).reshape(x.shape[0], n_tok)
    nk = n_feat // P

    # Stationary weights with the contraction split [k, p] mapped to
    # partitions: w_packed[p, k*M + m] = W[k*P + p, m].
    wq = np.asarray(W_q, dtype=np.float32).reshape(nk, P, DQ)
    wq_packed = np.ascontiguousarray(wq.transpose(1, 0, 2)).reshape(P, nk * DQ)
    wsm = np.concatenate(
        [np.asarray(W_scores, np.float32), np.asarray(W_mlp, np.float32)], axis=1
    ).reshape(nk, P, 2)
    wsm_packed = np.ascontiguousarray(wsm.transpose(1, 0, 2)).reshape(P, nk * 2)
    bq_packed = np.asarray(b_q, np.float32).reshape(DQ, 1)
    bsm_packed = np.array(
        [[np.float32(np.asarray(b_scores).reshape(-1)[0])],
         [np.float32(np.asarray(b_mlp).reshape(-1)[0])]], dtype=np.float32
    )
    mneg = np.where(mask, np.float32(NEG_BIG), np.float32(0.0)).astype(np.float32)

    n_cores = x.shape[0] // spc

    def one_core(c):
        sl = slice(c * spc, (c + 1) * spc)
        feats = x[sl, :, meta : meta + n_feat]          # [spc, n_tok, n_feat]
        xT = np.ascontiguousarray(feats.transpose(0, 2, 1))  # [spc, n_feat, n_tok]
        return {
            "xT": xT,
            "wq": wq_packed,
            "wsm": wsm_packed,
            "bq": bq_packed,
            "bsm": bsm_packed,
            "mneg": np.ascontiguousarray(mneg[sl]),
        }

    with _futures.ThreadPoolExecutor(max_workers=n_cores) as pool:
        return list(pool.map(one_core, range(n_cores)))


_CACHE = {}


def _get_nc():
    if "nc" not in _CACHE:
        _CACHE["nc"] = build_nc()
    return _CACHE["nc"]


def kernel(**inputs):
    nc, _ = _get_nc()
    in_maps = pack_core_inputs(**inputs)
    res = bass_utils.run_bass_kernel_spmd(nc, in_maps, core_ids=list(range(N_CORES)))
    out = np.concatenate(
        [np.asarray(res.results[c]["out"]).reshape(SPC) for c in range(N_CORES)]
    )
    return out.reshape(B, 1).astype(np.float32)


# revision 10
# speedup vs baseline: 106.7186x; 106.7186x over previous
"""DSMIL bag-of-tiles kernel for 8 Trainium2 NeuronCores (Bass/Tile).

Reference computation (per slide b of B=16, N=8192 tiles, F=2048 features):
    scores = feats @ W_scores + b_scores          (masked; max + argmax over N)
    queries = feats @ W_q + b_q
    attn    = softmax(queries @ queries[argmax])  (masked)
    bag     = sum(attn * (feats @ W_mlp + b_mlp))
    out[b]  = 0.5 * (max_score + bag)

Sharding: data-parallel over slides, 2 slides per core; small weights replicated.

Device-side layout: the PE contracts the partition dimension, so the feature
dimension must sit on partitions for every heavy matmul. The host hands each
core its feats pre-transposed as xT[slide, feature, token] f32; the device
then streams featsT chunks through the PE with the (replicated, repacked)
weights stationary. Everything on device is fp32, so max/argmax/softmax match
the fp32 reference exactly (no low-precision rescue logic needed).
"""

import os
import sys
import concurrent.futures as _futures

import numpy as np

for _p in ("/opt/trn_rl_repo", "/opt/pypackages"):
    if os.path.isdir(_p) and _p not in sys.path:
        sys.path.append(_p)

import concourse.bass as bass
import concourse.tile as tile
from concourse import bacc, mybir
from concourse import bass_utils

F32 = mybir.dt.float32
U32 = mybir.dt.uint32
AF = mybir.ActivationFunctionType
OP = mybir.AluOpType

# Problem constants (hardcoded per harness contract).
B, N, IN_FEAT, META, DQ = 16, 8192, 2048, 3, 128
N_CORES = 8
SPC = B // N_CORES          # slides per core = 2
P = 128                     # partitions / contraction tile
NEG_BIG = -1.0e30


def build_nc(n_tok=N, n_feat=IN_FEAT, spc=SPC, chunk=2048, ft_bufs=6, reps=1):
    """Build the per-core Bass program. Returns (nc, input name list).

    reps>1 repeats the whole computation in one NEFF (for timing isolation:
    wall(reps=R) - wall(reps=1) ~= (R-1) * T_kernel, amortizing dispatch).
    """
    nk = n_feat // P
    chunk = min(chunk, n_tok)
    n_chunks = n_tok // chunk
    sub = min(1024, n_tok)          # attn/softmax sub-chunk
    n_sub = n_tok // sub
    mm = 512                        # fp32 moving-operand limit / psum bank

    nc = bacc.Bacc("TRN2", target_bir_lowering=False, debug=False)

    xT_d = nc.dram_tensor("xT", [spc, n_feat, n_tok], F32, kind="ExternalInput").ap()
    wq_d = nc.dram_tensor("wq", [P, nk * DQ], F32, kind="ExternalInput").ap()
    wsm_d = nc.dram_tensor("wsm", [P, nk * 2], F32, kind="ExternalInput").ap()
    bq_d = nc.dram_tensor("bq", [P, 1], F32, kind="ExternalInput").ap()
    bsm_d = nc.dram_tensor("bsm", [2, 1], F32, kind="ExternalInput").ap()
    mneg_d = nc.dram_tensor("mneg", [spc, n_tok], F32, kind="ExternalInput").ap()
    out_d = nc.dram_tensor("out", [1, spc], F32, kind="ExternalOutput").ap()

    with tile.TileContext(nc) as tc:
        with (
            tc.tile_pool(name="consts", bufs=1) as cpool,
            tc.tile_pool(name="ft", bufs=ft_bufs) as ftpool,
            tc.tile_pool(name="qt", bufs=1) as qtpool,
            tc.tile_pool(name="rows", bufs=1) as rpool,
            tc.tile_pool(name="tiny", bufs=2) as tpool,
            tc.tile_pool(name="psq", bufs=1, space="PSUM") as psq,
            tc.tile_pool(name="psr", bufs=1, space="PSUM") as psr,
        ):
            wq_sb = cpool.tile([P, nk * DQ], F32)
            nc.sync.dma_start(wq_sb[:], wq_d[:])
            wsm_sb = cpool.tile([P, nk * 2], F32)
            nc.sync.dma_start(wsm_sb[:], wsm_d[:])
            bq_sb = cpool.tile([P, 1], F32)
            nc.sync.dma_start(bq_sb[:], bq_d[:])
            bsm_sb = cpool.tile([2, 1], F32)
            nc.sync.dma_start(bsm_sb[:], bsm_d[:])
            out_sb = cpool.tile([1, spc], F32)

            for s in [sl for _ in range(reps) for sl in range(spc)]:
                # scoresT/mlpT rows [2, n_tok] and queriesT [DQ, n_tok]
                m2 = rpool.tile([2, n_tok], F32, tag="m2", name=f"m2_{s}")
                qT = qtpool.tile([DQ, n_tok], F32, tag="qt", name=f"qT{s}")

                for c in range(n_chunks):
                    c0 = c * chunk
                    qp = psq.tile([DQ, chunk], F32, tag="qp", name=f"qp{s}_{c}")
                    sp = psr.tile([2, chunk], F32, tag="rowps", name=f"sp{s}_{c}")
                    for k in range(nk):
                        ft = ftpool.tile([P, chunk], F32, tag="ft", name=f"ft{s}_{c}_{k}")
                        nc.sync.dma_start(
                            ft[:], xT_d[s, k * P : (k + 1) * P, c0 : c0 + chunk]
                        )
                        st, sp_flag = (k == 0), (k == nk - 1)
                        for j in range(chunk // mm):
                            js = slice(j * mm, (j + 1) * mm)
                            nc.tensor.matmul(
                                qp[:, js],
                                wq_sb[:, k * DQ : (k + 1) * DQ],
                                ft[:, js],
                                start=st,
                                stop=sp_flag,
                            )
                            nc.tensor.matmul(
                                sp[:, js],
                                wsm_sb[:, k * 2 : (k + 1) * 2],
                                ft[:, js],
                                start=st,
                                stop=sp_flag,
                            )
                    # evacuate PSUM (+bias) on the scalar engine
                    nc.scalar.activation(
                        qT[:, c0 : c0 + chunk], qp[:], AF.Identity, bias=bq_sb[:], scale=1.0
                    )
                    nc.scalar.activation(
                        m2[:, c0 : c0 + chunk], sp[:], AF.Identity, bias=bsm_sb[:], scale=1.0
                    )

                # mask scores row in place (chunked), then global top-1 value
                for c in range(n_chunks):
                    cs = slice(c * chunk, (c + 1) * chunk)
                    mnc = tpool.tile([1, chunk], F32, tag="mnc", name=f"mnc{s}_{c}")
                    nc.sync.dma_start(mnc[:], mneg_d[s : s + 1, cs])
                    nc.vector.tensor_tensor(m2[0:1, cs], m2[0:1, cs], mnc[:], OP.add)
                m8 = tpool.tile([1, 8], F32, tag="m8", name=f"m8_{s}")
                nc.vector.max(m8[:], m2[0:1, :])

                # q_top[d] = sum_t qT[d, t] * onehot[t], via bcast-DMA + mult + reduce
                qparts = tpool.tile([DQ, n_chunks], F32, tag="qparts", name=f"qparts{s}")
                for c in range(n_chunks):
                    cs = slice(c * chunk, (c + 1) * chunk)
                    hc = tpool.tile([1, chunk], F32, tag="hc", name=f"hc{s}_{c}")
                    nc.vector.tensor_scalar(
                        hc[:], m2[0:1, cs], m8[0:1, 0:1], None, OP.is_equal
                    )
                    hb = tpool.tile([DQ, chunk], F32, tag="hb", name=f"hb{s}_{c}")
                    nc.gpsimd.partition_broadcast(hb[:], hc[:], channels=DQ)
                    nc.vector.tensor_tensor(hb[:], hb[:], qT[:, cs], OP.mult)
                    nc.vector.tensor_reduce(
                        qparts[:, c : c + 1], hb[:], axis=mybir.AxisListType.X, op=OP.add
                    )
                qtop = tpool.tile([DQ, 1], F32, tag="qtop", name=f"qtop{s}")
                nc.vector.tensor_reduce(
                    qtop[:], qparts[:], axis=mybir.AxisListType.X, op=OP.add
                )

                # softmax reference point M0 = q_top . q_top
                m0p = psr.tile([1, 1], F32, tag="rowps", name=f"m0p{s}")
                nc.tensor.matmul(m0p[0:1, 0:1], qtop[:], qtop[:])
                negM0 = tpool.tile([1, 1], F32, tag="negm0", name=f"negM0_{s}")
                nc.vector.tensor_scalar(negM0[:], m0p[0:1, 0:1], -1.0, None, OP.mult)

                zparts = tpool.tile([1, n_sub], F32, tag="zparts", name=f"zparts{s}")
                sparts = tpool.tile([1, n_sub], F32, tag="sparts", name=f"sparts{s}")
                for q in range(n_sub):
                    q0 = q * sub
                    qs = slice(q0, q0 + sub)
                    atp = psr.tile([1, sub], F32, tag="rowps", name=f"atp{s}_{q}")
                    for j in range(sub // mm):
                        js = slice(j * mm, (j + 1) * mm)
                        nc.tensor.matmul(
                            atp[0:1, js], qtop[:], qT[:, q0 + j * mm : q0 + (j + 1) * mm]
                        )
                    mnq = tpool.tile([1, sub], F32, tag="mnq", name=f"mnq{s}_{q}")
                    nc.sync.dma_start(mnq[:], mneg_d[s : s + 1, qs])
                    al = tpool.tile([1, sub], F32, tag="al", name=f"al{s}_{q}")
                    nc.vector.scalar_tensor_tensor(
                        al[:], atp[0:1, :], 0.0, mnq[:], OP.add, OP.add
                    )
                    ex = tpool.tile([1, sub], F32, tag="ex", name=f"ex{s}_{q}")
                    nc.scalar.activation(ex[:], al[:], AF.Exp, bias=negM0[:], scale=1.0)
                    nc.vector.tensor_reduce(
                        zparts[0:1, q : q + 1], ex[:], axis=mybir.AxisListType.X, op=OP.add
                    )
                    mc = tpool.tile([1, sub], F32, tag="mc", name=f"mc{s}_{q}")
                    nc.sync.dma_start(mc[:], m2[1:2, qs])
                    nc.vector.tensor_tensor(ex[:], ex[:], mc[:], OP.mult)
                    nc.vector.tensor_reduce(
                        sparts[0:1, q : q + 1], ex[:], axis=mybir.AxisListType.X, op=OP.add
                    )

                zz = tpool.tile([1, 1], F32, tag="zz", name=f"zz{s}")
                nc.vector.tensor_reduce(zz[:], zparts[:], axis=mybir.AxisListType.X, op=OP.add)
                ss = tpool.tile([1, 1], F32, tag="ss", name=f"ss{s}")
                nc.vector.tensor_reduce(ss[:], sparts[:], axis=mybir.AxisListType.X, op=OP.add)
                rz = tpool.tile([1, 1], F32, tag="rz", name=f"rz{s}")
                nc.vector.reciprocal(rz[:], zz[:])
                bag = tpool.tile([1, 1], F32, tag="bag", name=f"bag{s}")
                nc.vector.tensor_tensor(bag[:], ss[:], rz[:], OP.mult)
                tot = tpool.tile([1, 1], F32, tag="tot", name=f"tot{s}")
                nc.vector.tensor_tensor(tot[:], bag[:], m8[0:1, 0:1], OP.add)
                nc.vector.tensor_scalar(out_sb[0:1, s : s + 1], tot[:], 0.5, None, OP.mult)

            nc.sync.dma_start(out_d[:], out_sb[:])

    nc.compile()
    in_names = ["xT", "wq", "wsm", "bq", "bsm", "mneg"]
    return nc, in_names


def pack_core_inputs(x, mask, W_scores, b_scores, W_q, b_q, W_mlp, b_mlp,
                     n_tok=N, n_feat=IN_FEAT, spc=SPC, meta=META):
    """Host-side shard + repack: per-core input dicts for run_bass_kernel_spmd."""
    x = np.asarray(x, dtype=np.float32)
    mask = np.asarray(mask).astype(bool).reshape(x.shape[0], n_tok)
    nk = n_feat // P

    # Stationary weights with the contraction split [k, p] mapped to
    # partitions: w_packed[p, k*M + m] = W[k*P + p, m].
    wq = np.asarray(W_q, dtype=np.float32).reshape(nk, P, DQ)
    wq_packed = np.ascontiguousarray(wq.transpose(1, 0, 2)).reshape(P, nk * DQ)
    wsm = np.concatenate(
        [np.asarray(W_scores, np.float32), np.asarray(W_mlp, np.float32)], axis=1
    ).reshape(nk, P, 2)
    wsm_packed = np.ascontiguousarray(wsm.transpose(1, 0, 2)).reshape(P, nk * 2)
    bq_packed = np.asarray(b_q, np.float32).reshape(DQ, 1)
    bsm_packed = np.array(
        [[np.float32(np.asarray(b_scores).reshape(-1)[0])],
         [np.float32(np.asarray(b_mlp).reshape(-1)[0])]], dtype=np.float32
    )
    mneg = np.where(mask, np.float32(NEG_BIG), np.float32(0.0)).astype(np.float32)

    n_cores = x.shape[0] // spc

    def one_core(c):
        sl = slice(c * spc, (c + 1) * spc)
        feats = x[sl, :, meta : meta + n_feat]          # [spc, n_tok, n_feat]
        xT = np.ascontiguousarray(feats.transpose(0, 2, 1))  # [spc, n_feat, n_tok]
        return {
            "xT": xT,
            "wq": wq_packed,
            "wsm": wsm_packed,
            "bq": bq_packed,
            "bsm": bsm_packed,
            "mneg": np.ascontiguousarray(mneg[sl]),
        }

    with _futures.ThreadPoolExecutor(max_workers=n_cores) as pool:
        return list(pool.map(one_core, range(n_cores)))


_CACHE = {}


def _get_nc():
    if "nc" not in _CACHE:
        _CACHE["nc"] = build_nc()
    return _CACHE["nc"]


def kernel(**inputs):
    nc, _ = _get_nc()
    in_maps = pack_core_inputs(**inputs)
    res = bass_utils.run_bass_kernel_spmd(nc, in_maps, core_ids=list(range(N_CORES)))
    out = np.concatenate(
        [np.asarray(res.results[c]["out"]).reshape(SPC) for c in range(N_CORES)]
    )
    return out.reshape(B, 1).astype(np.float32)


# revision 13
# speedup vs baseline: 149.9241x; 1.4049x over previous
"""DSMIL bag-of-tiles kernel for 8 Trainium2 NeuronCores (Bass/Tile).

Reference computation (per slide b of B=16, N=8192 tiles, F=2048 features):
    scores = feats @ W_scores + b_scores          (masked; max + argmax over N)
    queries = feats @ W_q + b_q
    attn    = softmax(queries @ queries[argmax])  (masked)
    bag     = sum(attn * (feats @ W_mlp + b_mlp))
    out[b]  = 0.5 * (max_score + bag)

Sharding: data-parallel over slides, 2 slides per core; small weights replicated.

Device-side layout: the PE contracts the partition dimension, so the feature
dimension must sit on partitions for every heavy matmul. The host hands each
core its feats pre-transposed as xT[slide, feature, token] f32; the device
then streams featsT chunks through the PE with the (replicated, repacked)
weights stationary. Everything on device is fp32, so max/argmax/softmax match
the fp32 reference exactly (no low-precision rescue logic needed).
"""

import os
import sys
import concurrent.futures as _futures

import numpy as np

for _p in ("/opt/trn_rl_repo", "/opt/pypackages"):
    if os.path.isdir(_p) and _p not in sys.path:
        sys.path.append(_p)

import concourse.bass as bass
import concourse.tile as tile
from concourse import bacc, mybir
from concourse import bass_utils

F32 = mybir.dt.float32
F32R = mybir.dt.float32r
U32 = mybir.dt.uint32
AF = mybir.ActivationFunctionType
OP = mybir.AluOpType

# Problem constants (hardcoded per harness contract).
B, N, IN_FEAT, META, DQ = 16, 8192, 2048, 3, 128
N_CORES = 8
SPC = B // N_CORES          # slides per core = 2
P = 128                     # partitions / contraction tile
NEG_BIG = -1.0e30


def build_nc(n_tok=N, n_feat=IN_FEAT, spc=SPC, chunk=2048, ft_bufs=6, reps=1):
    """Build the per-core Bass program. Returns (nc, input name list).

    reps>1 repeats the whole computation in one NEFF (for timing isolation:
    wall(reps=R) - wall(reps=1) ~= (R-1) * T_kernel, amortizing dispatch).
    """
    nk = n_feat // P
    chunk = min(chunk, n_tok)
    n_chunks = n_tok // chunk
    sub = min(1024, n_tok)          # attn/softmax sub-chunk
    n_sub = n_tok // sub
    mm = 512                        # fp32 moving-operand limit / psum bank

    nc = bacc.Bacc("TRN2", target_bir_lowering=False, debug=False)

    xT_d = nc.dram_tensor("xT", [spc, n_feat, n_tok], F32R, kind="ExternalInput").ap()
    wq_d = nc.dram_tensor("wq", [P, nk * DQ], F32R, kind="ExternalInput").ap()
    wsm_d = nc.dram_tensor("wsm", [P, nk * 2], F32R, kind="ExternalInput").ap()
    bq_d = nc.dram_tensor("bq", [P, 1], F32, kind="ExternalInput").ap()
    bsm_d = nc.dram_tensor("bsm", [2, 1], F32, kind="ExternalInput").ap()
    mneg_d = nc.dram_tensor("mneg", [spc, n_tok], F32, kind="ExternalInput").ap()
    out_d = nc.dram_tensor("out", [1, spc], F32, kind="ExternalOutput").ap()

    with tile.TileContext(nc) as tc:
        with (
            tc.tile_pool(name="consts", bufs=1) as cpool,
            tc.tile_pool(name="ft", bufs=ft_bufs) as ftpool,
            tc.tile_pool(name="qt", bufs=1) as qtpool,
            tc.tile_pool(name="rows", bufs=1) as rpool,
            tc.tile_pool(name="tiny", bufs=2) as tpool,
            tc.tile_pool(name="psq", bufs=1, space="PSUM") as psq,
            tc.tile_pool(name="psr", bufs=1, space="PSUM") as psr,
        ):
            wq_sb = cpool.tile([P, nk * DQ], F32R)
            nc.sync.dma_start(wq_sb[:], wq_d[:])
            wsm_sb = cpool.tile([P, nk * 2], F32R)
            nc.sync.dma_start(wsm_sb[:], wsm_d[:])
            bq_sb = cpool.tile([P, 1], F32)
            nc.sync.dma_start(bq_sb[:], bq_d[:])
            bsm_sb = cpool.tile([2, 1], F32)
            nc.sync.dma_start(bsm_sb[:], bsm_d[:])
            out_sb = cpool.tile([1, spc], F32)

            for s in [sl for _ in range(reps) for sl in range(spc)]:
                # scoresT/mlpT rows [2, n_tok] and queriesT [DQ, n_tok]
                m2 = rpool.tile([2, n_tok], F32, tag="m2", name=f"m2_{s}")
                qT = qtpool.tile([DQ, n_tok], F32, tag="qt", name=f"qT{s}")

                for c in range(n_chunks):
                    c0 = c * chunk
                    qp = psq.tile([DQ, chunk], F32, tag="qp", name=f"qp{s}_{c}")
                    sp = psr.tile([2, chunk], F32, tag="rowps", name=f"sp{s}_{c}")
                    for k in range(nk):
                        ft = ftpool.tile([P, chunk], F32R, tag="ft", name=f"ft{s}_{c}_{k}")
                        nc.sync.dma_start(
                            ft[:], xT_d[s, k * P : (k + 1) * P, c0 : c0 + chunk]
                        )
                        st, sp_flag = (k == 0), (k == nk - 1)
                        for j in range(chunk // mm):
                            js = slice(j * mm, (j + 1) * mm)
                            nc.tensor.matmul(
                                qp[:, js],
                                wq_sb[:, k * DQ : (k + 1) * DQ],
                                ft[:, js],
                                start=st,
                                stop=sp_flag,
                            )
                            nc.tensor.matmul(
                                sp[:, js],
                                wsm_sb[:, k * 2 : (k + 1) * 2],
                                ft[:, js],
                                start=st,
                                stop=sp_flag,
                            )
                    # evacuate PSUM (+bias) on the scalar engine
                    nc.scalar.activation(
                        qT[:, c0 : c0 + chunk], qp[:], AF.Identity, bias=bq_sb[:], scale=1.0
                    )
                    nc.scalar.activation(
                        m2[:, c0 : c0 + chunk], sp[:], AF.Identity, bias=bsm_sb[:], scale=1.0
                    )

                # mask scores row in place (chunked), then global top-1 value
                for c in range(n_chunks):
                    cs = slice(c * chunk, (c + 1) * chunk)
                    mnc = tpool.tile([1, chunk], F32, tag="mnc", name=f"mnc{s}_{c}")
                    nc.sync.dma_start(mnc[:], mneg_d[s : s + 1, cs])
                    nc.vector.tensor_tensor(m2[0:1, cs], m2[0:1, cs], mnc[:], OP.add)
                m8 = tpool.tile([1, 8], F32, tag="m8", name=f"m8_{s}")
                nc.vector.max(m8[:], m2[0:1, :])

                # q_top[d] = sum_t qT[d, t] * onehot[t], via bcast-DMA + mult + reduce
                qparts = tpool.tile([DQ, n_chunks], F32, tag="qparts", name=f"qparts{s}")
                for c in range(n_chunks):
                    cs = slice(c * chunk, (c + 1) * chunk)
                    hc = tpool.tile([1, chunk], F32, tag="hc", name=f"hc{s}_{c}")
                    nc.vector.tensor_scalar(
                        hc[:], m2[0:1, cs], m8[0:1, 0:1], None, OP.is_equal
                    )
                    hb = tpool.tile([DQ, chunk], F32, tag="hb", name=f"hb{s}_{c}")
                    nc.gpsimd.partition_broadcast(hb[:], hc[:], channels=DQ)
                    nc.vector.tensor_tensor(hb[:], hb[:], qT[:, cs], OP.mult)
                    nc.vector.tensor_reduce(
                        qparts[:, c : c + 1], hb[:], axis=mybir.AxisListType.X, op=OP.add
                    )
                qtop = tpool.tile([DQ, 1], F32, tag="qtop", name=f"qtop{s}")
                nc.vector.tensor_reduce(
                    qtop[:], qparts[:], axis=mybir.AxisListType.X, op=OP.add
                )

                # softmax reference point M0 = q_top . q_top
                m0p = psr.tile([1, 1], F32, tag="rowps", name=f"m0p{s}")
                nc.tensor.matmul(m0p[0:1, 0:1], qtop[:], qtop[:])
                negM0 = tpool.tile([1, 1], F32, tag="negm0", name=f"negM0_{s}")
                nc.vector.tensor_scalar(negM0[:], m0p[0:1, 0:1], -1.0, None, OP.mult)

                zparts = tpool.tile([1, n_sub], F32, tag="zparts", name=f"zparts{s}")
                sparts = tpool.tile([1, n_sub], F32, tag="sparts", name=f"sparts{s}")
                for q in range(n_sub):
                    q0 = q * sub
                    qs = slice(q0, q0 + sub)
                    atp = psr.tile([1, sub], F32, tag="rowps", name=f"atp{s}_{q}")
                    for j in range(sub // mm):
                        js = slice(j * mm, (j + 1) * mm)
                        nc.tensor.matmul(
                            atp[0:1, js], qtop[:], qT[:, q0 + j * mm : q0 + (j + 1) * mm]
                        )
                    mnq = tpool.tile([1, sub], F32, tag="mnq", name=f"mnq{s}_{q}")
                    nc.sync.dma_start(mnq[:], mneg_d[s : s + 1, qs])
                    al = tpool.tile([1, sub], F32, tag="al", name=f"al{s}_{q}")
                    nc.vector.scalar_tensor_tensor(
                        al[:], atp[0:1, :], 0.0, mnq[:], OP.add, OP.add
                    )
                    ex = tpool.tile([1, sub], F32, tag="ex", name=f"ex{s}_{q}")
                    nc.scalar.activation(ex[:], al[:], AF.Exp, bias=negM0[:], scale=1.0)
                    nc.vector.tensor_reduce(
                        zparts[0:1, q : q + 1], ex[:], axis=mybir.AxisListType.X, op=OP.add
                    )
                    mc = tpool.tile([1, sub], F32, tag="mc", name=f"mc{s}_{q}")
                    nc.sync.dma_start(mc[:], m2[1:2, qs])
                    nc.vector.tensor_tensor(ex[:], ex[:], mc[:], OP.mult)
                    nc.vector.tensor_reduce(
                        sparts[0:1, q : q + 1], ex[:], axis=mybir.AxisListType.X, op=OP.add
                    )

                zz = tpool.tile([1, 1], F32, tag="zz", name=f"zz{s}")
                nc.vector.tensor_reduce(zz[:], zparts[:], axis=mybir.AxisListType.X, op=OP.add)
                ss = tpool.tile([1, 1], F32, tag="ss", name=f"ss{s}")
                nc.vector.tensor_reduce(ss[:], sparts[:], axis=mybir.AxisListType.X, op=OP.add)
                rz = tpool.tile([1, 1], F32, tag="rz", name=f"rz{s}")
                nc.vector.reciprocal(rz[:], zz[:])
                bag = tpool.tile([1, 1], F32, tag="bag", name=f"bag{s}")
                nc.vector.tensor_tensor(bag[:], ss[:], rz[:], OP.mult)
                tot = tpool.tile([1, 1], F32, tag="tot", name=f"tot{s}")
                nc.vector.tensor_tensor(tot[:], bag[:], m8[0:1, 0:1], OP.add)
                nc.vector.tensor_scalar(out_sb[0:1, s : s + 1], tot[:], 0.5, None, OP.mult)

            nc.sync.dma_start(out_d[:], out_sb[:])

    nc.compile()
    in_names = ["xT", "wq", "wsm", "bq", "bsm", "mneg"]
    return nc, in_names


def round_fp32r(a):
    """RNE-round fp32 to the PE's fp32r format (11-bit mantissa, IEEE layout)."""
    u = np.ascontiguousarray(a, dtype=np.float32).view(np.uint32)
    low = u & np.uint32(0xFFF)
    base = u & ~np.uint32(0xFFF)
    lsb = (u >> np.uint32(12)) & np.uint32(1)
    inc = (low > np.uint32(0x800)) | ((low == np.uint32(0x800)) & (lsb == 1))
    return (base + np.where(inc, np.uint32(0x1000), np.uint32(0))).view(np.float32)


def pack_core_inputs(x, mask, W_scores, b_scores, W_q, b_q, W_mlp, b_mlp,
                     n_tok=N, n_feat=IN_FEAT, spc=SPC, meta=META):
    """Host-side shard + repack: per-core input dicts for run_bass_kernel_spmd."""
    x = np.asarray(x, dtype=np.float32)
    mask = np.asarray(mask).astype(bool).reshape(x.shape[0], n_tok)
    nk = n_feat // P

    # Stationary weights with the contraction split [k, p] mapped to
    # partitions: w_packed[p, k*M + m] = W[k*P + p, m].
    wq = np.asarray(W_q, dtype=np.float32).reshape(nk, P, DQ)
    wq_packed = round_fp32r(
        np.ascontiguousarray(wq.transpose(1, 0, 2)).reshape(P, nk * DQ))
    wsm = np.concatenate(
        [np.asarray(W_scores, np.float32), np.asarray(W_mlp, np.float32)], axis=1
    ).reshape(nk, P, 2)
    wsm_packed = round_fp32r(
        np.ascontiguousarray(wsm.transpose(1, 0, 2)).reshape(P, nk * 2))
    bq_packed = np.asarray(b_q, np.float32).reshape(DQ, 1)
    bsm_packed = np.array(
        [[np.float32(np.asarray(b_scores).reshape(-1)[0])],
         [np.float32(np.asarray(b_mlp).reshape(-1)[0])]], dtype=np.float32
    )
    mneg = np.where(mask, np.float32(NEG_BIG), np.float32(0.0)).astype(np.float32)

    n_cores = x.shape[0] // spc

    def one_core(c):
        sl = slice(c * spc, (c + 1) * spc)
        feats = x[sl, :, meta : meta + n_feat]          # [spc, n_tok, n_feat]
        xT = round_fp32r(np.ascontiguousarray(feats.transpose(0, 2, 1)))
        return {
            "xT": xT,
            "wq": wq_packed,
            "wsm": wsm_packed,
            "bq": bq_packed,
            "bsm": bsm_packed,
            "mneg": np.ascontiguousarray(mneg[sl]),
        }

    with _futures.ThreadPoolExecutor(max_workers=n_cores) as pool:
        return list(pool.map(one_core, range(n_cores)))


_CACHE = {}


def _get_nc():
    if "nc" not in _CACHE:
        _CACHE["nc"] = build_nc()
    return _CACHE["nc"]


def kernel(**inputs):
    nc, _ = _get_nc()
    in_maps = pack_core_inputs(**inputs)
    res = bass_utils.run_bass_kernel_spmd(nc, in_maps, core_ids=list(range(N_CORES)))
    out = np.concatenate(
        [np.asarray(res.results[c]["out"]).reshape(SPC) for c in range(N_CORES)]
    )
    return out.reshape(B, 1).astype(np.float32)
